# revision 27
# baseline (speedup 1.0000x reference)
# Trainium2 Bass kernel for nn_GATv2_52450140619486.
#
# Math (validated vs reference to 7e-8 in a float64 golden model):
#   - 3-layer input MLP, 5 stacked GATv2Conv(64,64,heads=7,concat=False),
#     3-layer output MLP, log_softmax.
#   - GATv2 attention decomposed into dense matmuls + gathers:
#         att*lrelu(z) = 0.6*att*z + 0.4*sign(att)*|att*z|
#     with per-head channel sign-permutation done on the host, so the |y|
#     group sums are one strided tensor_reduce per edge tile.
#   - Softmax per dst node without max subtraction (|logits| <= 1.5).
#   - Weighted segment-sum over edges = TensorE matmul with host-built 0/1
#     one-hot S tiles (edges sorted by dst); alpha rides the rhs as the outer
#     product a[e,h] * h_src[e,k]; per-head output projection and head-mean
#     fold into the dst-side matmul.
#
# Sharding: nodes range-partitioned across 8 cores by dst; each core gathers
# source-node features from a replicated node table, AllGathered per layer.
import numpy as np

P = 128          # partitions / edge-tile size / dst-window size
STS = 16         # edge tiles per supertile (one dma_gather batch)
NC_DEFAULT = 8

_CACHE = {}


# ---------------------------------------------------------------------------
# Host-side preparation (numpy; indices / one-hot / weight folding only)
# ---------------------------------------------------------------------------

def _wrap_idx(ids):
    """indirect_dma_start offset layout: [128, n/128] int32 with
    idx[p, g] = ids[g*128 + p] (edge e -> partition e%128, block e//128)."""
    n = len(ids)
    assert n % P == 0
    return np.ascontiguousarray(
        ids.reshape(n // P, P).T.astype(np.int32))


def _prep_edges(src, dst, npc, nc):
    """Per-core edge tiles; per-window tile counts equalized across cores so
    one NEFF serves all cores. Edges already have self-loops appended."""
    nw = (npc + P - 1) // P
    order = np.argsort(dst, kind="stable")
    src = src[order]
    dst = dst[order]
    cores = []
    for r in range(nc):
        lo, hi = r * npc, (r + 1) * npc
        m0, m1 = np.searchsorted(dst, lo), np.searchsorted(dst, hi)
        s_r, d_r = src[m0:m1], dst[m0:m1]
        win = []
        for w in range(nw):
            wlo, whi = lo + w * P, min(lo + (w + 1) * P, hi)
            e0, e1 = np.searchsorted(d_r, wlo), np.searchsorted(d_r, whi)
            win.append((s_r[e0:e1].copy(), (d_r[e0:e1] - wlo).copy()))
        cores.append(win)

    # dummy edges needed to give the out-of-range dst slots of the last
    # window a nonzero softmax denominator
    n_empty = nw * P - npc

    # per-window tile counts, equalized across cores
    tpw = []
    for w in range(nw):
        need = max(len(cores[r][w][0]) + (n_empty if w == nw - 1 else 0)
                   for r in range(nc))
        tpw.append((need + P - 1) // P)

    out = []
    for r in range(nc):
        src_ids, dst_ids, S_list, win_of = [], [], [], []
        for w in range(nw):
            es, ed = cores[r][w]
            valid = np.ones(len(es), bool)
            if w == nw - 1 and n_empty:
                # dummies covering empty dst slots (local ids npc%P .. 127)
                base = npc - (nw - 1) * P
                es = np.concatenate([es, np.zeros(n_empty, np.int64)])
                ed = np.concatenate([ed, base + np.arange(n_empty)])
                valid = np.concatenate([valid, np.ones(n_empty, bool)])
            pad = tpw[w] * P - len(es)
            assert pad >= 0
            es = np.concatenate([es, np.zeros(pad, np.int64)])
            ed = np.concatenate([ed, np.zeros(pad, np.int64)])
            valid = np.concatenate([valid, np.zeros(pad, bool)])
            for i in range(tpw[w]):
                sl = slice(i * P, (i + 1) * P)
                S = np.zeros((P, P), np.float32)
                ee = np.arange(P)[valid[sl]]
                S[ee, ed[sl][valid[sl]]] = 1.0
                src_ids.append(es[sl])
                dst_ids.append(r * npc + w * P + ed[sl])
                S_list.append(S)
                win_of.append(w)
        n_t = len(win_of)
        n_st = (n_t + STS - 1) // STS
        sid = np.concatenate(src_ids)
        pad = n_st * STS * P - len(sid)
        sid = np.concatenate([sid, np.zeros(pad, np.int64)])
        srcw = np.stack([_wrap_idx(sid[i * STS * P:(i + 1) * STS * P])
                         for i in range(n_st)])
        S = np.stack(S_list)
        out.append(dict(srcw=srcw, S=S,
                        S01T=np.ascontiguousarray(S.transpose(0, 2, 1)),
                        win_of=win_of, n_tiles=n_t, n_st=n_st, nw=nw))
    return out


def _prep_gat_weights(Wl, bl, Wr, att, cbias, H, C):
    """Fold att (sign permutation + 0.4/0.6 split) into y weights; build the
    dst-side projection. float64 folding, cast at the end by caller."""
    L, K = Wl.shape[0], Wl.shape[1]
    Wl = np.asarray(Wl, np.float64).reshape(L, K, H, C)
    Wr = np.asarray(Wr, np.float64).reshape(L, K, H, C)
    bl = np.asarray(bl, np.float64).reshape(L, H, C)
    att = np.asarray(att, np.float64)

    Gm = 0
    for l in range(L):
        for h in range(H):
            Gm = max(Gm, int((att[l, h] >= 0).sum()), int((att[l, h] < 0).sum()))
    YC = 2 * H * Gm + H

    wy_hs = np.zeros((L, K + 1, YC))
    wy_hd = np.zeros((L, K + 1, YC))
    for l in range(L):
        for h in range(H):
            for gi, idxs in enumerate([np.where(att[l, h] >= 0)[0],
                                       np.where(att[l, h] < 0)[0]]):
                w04 = 0.4 * np.abs(att[l, h, idxs])
                c0 = (h * 2 + gi) * Gm
                wy_hs[l, :K, c0:c0 + len(idxs)] = Wl[l, :, h, idxs].T * w04
                wy_hd[l, :K, c0:c0 + len(idxs)] = Wr[l, :, h, idxs].T * w04
                wy_hd[l, K, c0:c0 + len(idxs)] = bl[l, h, idxs] * w04
            wy_hs[l, :K, 2 * H * Gm + h] = 0.6 * (Wl[l, :, h, :] @ att[l, h])
            wy_hd[l, :K, 2 * H * Gm + h] = 0.6 * (Wr[l, :, h, :] @ att[l, h])
            wy_hd[l, K, 2 * H * Gm + h] = 0.6 * (bl[l, h] @ att[l, h])

    # dst side: out[d, c] = sum_{h,k} aggn[d, h*K+k] * W[h*K+k, c] + crow[c]
    HK = H * K
    nch = (HK + P - 1) // P
    wbig = np.zeros((L, P, nch, C))          # [l, row%P, row//P, c]
    for l in range(L):
        for h in range(H):
            for k in range(K):
                row = h * K + k
                wbig[l, row % P, row // P, :] = Wl[l, k, h, :] / H
    crow = bl.mean(1) + np.asarray(cbias, np.float64)  # [L, C]
    return dict(Gm=Gm, YC=YC, wy_hs=wy_hs, wy_hd=wy_hd, wbig=wbig, crow=crow)


def _host_prep(inputs, nc_cores=NC_DEFAULT):
    import ml_dtypes
    bf16 = ml_dtypes.bfloat16

    x = np.asarray(inputs["x"], np.float32)
    ei = np.asarray(inputs["edge_index"])
    N, IN = x.shape
    H, C = np.asarray(inputs["att"]).shape[1:]
    npc = N // nc_cores
    loops = np.arange(N, dtype=np.int64)
    src = np.concatenate([ei[0].astype(np.int64), loops])
    dst = np.concatenate([ei[1].astype(np.int64), loops])

    edges = _prep_edges(src, dst, npc, nc_cores)
    gw = _prep_gat_weights(np.asarray(inputs["Wl"]), np.asarray(inputs["bl"]),
                           np.asarray(inputs["Wr"]), np.asarray(inputs["att"]),
                           np.asarray(inputs["cbias"]), H, C)

    W1 = np.asarray(inputs["W1"], np.float64)
    W2 = np.asarray(inputs["W2"], np.float64)
    W3 = np.asarray(inputs["W3"], np.float64)
    D1, D2, D3 = W1.shape[1], W2.shape[1], W3.shape[1]
    b1c = np.ascontiguousarray(
        np.asarray(inputs["b1"], np.float32).reshape(D1 // P, P).T)
    b2c = np.ascontiguousarray(
        np.asarray(inputs["b2"], np.float32).reshape(D2 // P, P).T)
    b3c = np.asarray(inputs["b3"], np.float32).reshape(D3, 1)

    def aug(V, c):
        V = np.asarray(V, np.float64)
        return np.concatenate([V, np.asarray(c, np.float64)[None, :]], 0)

    V1a, V2a, V3a = (aug(inputs["V1"], inputs["c1"]),
                     aug(inputs["V2"], inputs["c2"]),
                     aug(inputs["V3"], inputs["c3"]))

    shared = {
        "w1": W1.astype(bf16), "w2": W2.astype(bf16), "w3": W3.astype(bf16),
        "b1c": b1c, "b2c": b2c, "b3c": b3c,
        "wy_hs": gw["wy_hs"].astype(bf16), "wy_hd": gw["wy_hd"].astype(bf16),
        "wbig": gw["wbig"].astype(bf16),
        "crow": gw["crow"].astype(bf16),
        "v1a": V1a.astype(bf16), "v2a": V2a.astype(bf16),
        "v3a": V3a.astype(bf16),
    }
    in_maps = []
    for r in range(nc_cores):
        m = dict(shared)
        m["xt"] = np.ascontiguousarray(
            x[r * npc:(r + 1) * npc].T).astype(bf16)
        m["srcw"] = edges[r]["srcw"]
        m["s_all"] = edges[r]["S"].astype(bf16)
        m["s01t_all"] = edges[r]["S01T"].astype(bf16)
        in_maps.append(m)

    meta = dict(N=N, IN=IN, H=H, C=C, npc=npc, nc=nc_cores,
                Gm=gw["Gm"], YC=gw["YC"], D1=D1, D2=D2, D3=D3,
                O1=V1a.shape[1], O2=V2a.shape[1], O3=V3a.shape[1],
                n_st=edges[0]["n_st"], n_tiles=edges[0]["n_tiles"],
                win_of=tuple(edges[0]["win_of"]), nw=edges[0]["nw"],
                L=gw["wy_hs"].shape[0])
    for r in range(1, nc_cores):
        assert tuple(edges[r]["win_of"]) == meta["win_of"]
    return in_maps, meta


# ---------------------------------------------------------------------------
# Device program
# ---------------------------------------------------------------------------

def _build(meta):
    import concourse.tile as tile
    import concourse.mybir as mybir
    from concourse import bacc
    from concourse.bass import ts, ds
    from concourse.masks import make_identity

    import concourse.bass as bass

    f32 = mybir.dt.float32
    bf16 = mybir.dt.bfloat16
    i32 = mybir.dt.int32
    AF = mybir.ActivationFunctionType
    OP = mybir.AluOpType
    AX = mybir.AxisListType

    N, IN, H, C = meta["N"], meta["IN"], meta["H"], meta["C"]
    npc, NCC = meta["npc"], meta["nc"]
    Gm, YC = meta["Gm"], meta["YC"]
    D1, D2, D3 = meta["D1"], meta["D2"], meta["D3"]
    O1, O2, O3 = meta["O1"], meta["O2"], meta["O3"]
    L = meta["L"]
    n_st, n_tiles, win_of, nw = (meta["n_st"], meta["n_tiles"],
                                 meta["win_of"], meta["nw"])
    K1 = C + 1                    # 65: h dims + ones row
    NG = 2 * H                    # 14 sign groups
    Y0G = NG - 2                  # groups handled in bank 0
    Y0 = Y0G * Gm
    Y1C = YC - Y0                 # 2*Gm + H
    HK = H * C
    AGG = HK + H                  # 455
    NCH = (HK + P - 1) // P       # aggn k-chunks (4)
    assert Y0 <= 512 and Y1C <= 512 and AGG <= 512

    nc = bacc.Bacc("TRN2", target_bir_lowering=False, debug=False,
                   num_devices=NCC)

    def inp(name, shape, dtype):
        return nc.dram_tensor(name, list(shape), dtype,
                              kind="ExternalInput").ap()

    xt = inp("xt", (IN, npc), bf16)
    w1 = inp("w1", (IN, D1), bf16)
    w2 = inp("w2", (D1, D2), bf16)
    w3 = inp("w3", (D2, D3), bf16)
    b1c_d = inp("b1c", (P, D1 // P), f32)
    b2c_d = inp("b2c", (P, D2 // P), f32)
    b3c_d = inp("b3c", (D3, 1), f32)
    wy_hs_in = inp("wy_hs", (L, K1, YC), bf16)
    wy_hd_in = inp("wy_hd", (L, K1, YC), bf16)
    wbig_in = inp("wbig", (L, P, NCH, C), bf16)
    crow_in = inp("crow", (L, C), bf16)
    v1a = inp("v1a", (K1, O1), bf16)
    v2a = inp("v2a", (O1 + 1, O2), bf16)
    v3a = inp("v3a", (O2 + 1, O3), bf16)
    srcw_in = inp("srcw", (n_st, P, STS), i32)
    s_in = inp("s_all", (n_tiles, P, P), bf16)
    s01t_in = inp("s01t_all", (n_tiles, P, P), bf16)
    out_dram = nc.dram_tensor("out", [npc, O3], f32, kind="ExternalOutput").ap()

    from contextlib import ExitStack
    with tile.TileContext(nc) as tc, ExitStack() as _ctx:
        def pool(name, bufs, space="SBUF"):
            return _ctx.enter_context(
                tc.tile_pool(name=name, bufs=bufs, space=space))

        dram = pool("dram", 1, "DRAM")
        consts = pool("consts", 1)
        wpool = pool("wpool", 1)
        mlpw = pool("mlpw", 3)
        mlpx = pool("mlpx", 10)
        mlph = pool("mlph", 1)
        idxp = pool("idxp", 6)
        gath = pool("gath", 2)
        lhsp = pool("lhsp", 8)
        spool = pool("spool", 8)
        small = pool("small", 16)
        wtp = pool("wtp", 6)
        hpool = pool("hpool", 6)
        aypool = pool("aypool", 6)
        aggn_p = pool("aggn_p", 2)
        psum_y0 = pool("psum_y0", 2, "PSUM")
        psum_y1 = pool("psum_y1", 2, "PSUM")
        psum_agg = pool("psum_agg", 2, "PSUM")
        psum_t = pool("psum_t", 1, "PSUM")
        psum_m = pool("psum_m", 1, "PSUM")

        hloc = dram.tile([npc, P], bf16)
        hg = dram.tile([N, P], bf16)

        ident = consts.tile([P, P], bf16, tag="ident")
        make_identity(nc, ident[:])
        ones_row = consts.tile([1, P], bf16, tag="ones")
        nc.vector.memset(ones_row[:], 1.0)
        b1c = consts.tile([P, D1 // P], f32, tag="b1c")
        nc.sync.dma_start(b1c[:], b1c_d)
        b2c = consts.tile([P, D2 // P], f32, tag="b2c")
        nc.sync.dma_start(b2c[:], b2c_d)
        b3c = consts.tile([D3, 1], f32, tag="b3c")
        nc.sync.dma_start(b3c[:], b3c_d)

        wy_hs, wy_hd, wbig, crow = [], [], [], []
        for l in range(L):
            a = wpool.tile([K1, YC], bf16, tag=f"wyhs{l}")
            nc.sync.dma_start(a[:], wy_hs_in[l])
            wy_hs.append(a)
            b = wpool.tile([K1, YC], bf16, tag=f"wyhd{l}")
            nc.sync.dma_start(b[:], wy_hd_in[l])
            wy_hd.append(b)
            wb = wpool.tile([P, NCH, C], bf16, tag=f"wbig{l}")
            nc.sync.dma_start(wb[:], wbig_in[l])
            wbig.append(wb)
            cr = wpool.tile([1, C], bf16, tag=f"crow{l}")
            nc.sync.dma_start(cr[:], crow_in[l].unsqueeze(0))
            crow.append(cr)
        v1t = wpool.tile([K1, O1], bf16, tag="v1")
        nc.sync.dma_start(v1t[:], v1a)
        v2t = wpool.tile([O1 + 1, O2], bf16, tag="v2")
        nc.sync.dma_start(v2t[:], v2a)
        v3t = wpool.tile([O2 + 1, O3], bf16, tag="v3")
        nc.sync.dma_start(v3t[:], v3a)

        # ------------------------------------------------------------------
        def write_h(w, outp):
            """relu + store one [128,64] f32 psum tile into hloc rows."""
            hb = hpool.tile([P, P], bf16, tag="hb")
            nc.scalar.activation(hb[:, 0:C], outp[:], AF.Relu)
            nc.vector.memset(hb[:, C:P], 1.0)  # col C = lhsT ones row
            sz = min(P, npc - P * w)
            nc.sync.dma_start(hloc[P * w:P * w + sz, :], hb[0:sz, :])

        # ------------------------------------------------------------------
        def input_mlp():
            nK1, nK2, nK3 = IN // P, D1 // P, D2 // P
            ntiles = [(i * 512, min(512, npc - i * 512))
                      for i in range((npc + 511) // 512)]
            h0 = mlph.tile([P, nK2, npc], bf16, tag="h0t")
            h1 = mlph.tile([P, nK3, npc], bf16, tag="h1t")
            h2 = mlph.tile([D3, npc], bf16, tag="h2t")
            for (n0, nsz) in ntiles:
                xts = []
                for k in range(nK1):
                    xk = mlpx.tile([P, 512], bf16, tag="xts")
                    nc.sync.dma_start(xk[:, 0:nsz], xt[ts(k, P), ds(n0, nsz)])
                    xts.append(xk)
                for j in range(nK2):
                    ps = psum_m.tile([P, 512], f32, tag="m")
                    for k in range(nK1):
                        wt1 = mlpw.tile([P, P], bf16, tag="w1t")
                        nc.sync.dma_start(wt1[:], w1[ts(k, P), ts(j, P)])
                        nc.tensor.matmul(ps[:, 0:nsz], wt1[:], xts[k][:, 0:nsz],
                                         start=(k == 0), stop=(k == nK1 - 1))
                    nc.scalar.activation(h0[:, j, ds(n0, nsz)], ps[:, 0:nsz],
                                         AF.Relu, bias=b1c[:, j:j + 1])
                for j in range(nK3):
                    ps = psum_m.tile([P, 512], f32, tag="m")
                    for k in range(nK2):
                        wt2 = mlpw.tile([P, P], bf16, tag="w2t")
                        nc.sync.dma_start(wt2[:], w2[ts(k, P), ts(j, P)])
                        nc.tensor.matmul(ps[:, 0:nsz], wt2[:],
                                         h0[:, k, ds(n0, nsz)],
                                         start=(k == 0), stop=(k == nK2 - 1))
                    nc.scalar.activation(h1[:, j, ds(n0, nsz)], ps[:, 0:nsz],
                                         AF.Relu, bias=b2c[:, j:j + 1])
                ps = psum_m.tile([P, 512], f32, tag="m")
                for k in range(nK3):
                    wt3 = mlpw.tile([P, D3], bf16, tag="w3t")
                    nc.sync.dma_start(wt3[:], w3[ts(k, P), :])
                    nc.tensor.matmul(ps[0:D3, 0:nsz], wt3[:],
                                     h1[:, k, ds(n0, nsz)],
                                     start=(k == 0), stop=(k == nK3 - 1))
                nc.scalar.activation(h2[:, ds(n0, nsz)], ps[0:D3, 0:nsz],
                                     AF.Relu, bias=b3c[:, 0:1])
            for w in range(nw):
                sz = min(P, npc - P * w)
                tp = psum_m.tile([P, P], bf16, tag="m")
                nc.tensor.transpose(tp[0:sz, 0:D3], h2[0:D3, ds(P * w, sz)],
                                    ident[0:D3, 0:D3])
                hb = hpool.tile([P, P], bf16, tag="hb")
                nc.scalar.copy(hb[0:sz, 0:C], tp[0:sz, 0:D3])
                nc.vector.memset(hb[:, C:P], 1.0)
                nc.sync.dma_start(hloc[P * w:P * w + sz, :], hb[0:sz, :])

        # ------------------------------------------------------------------
        def out_mlp(w, outp):
            h5 = hpool.tile([P, C], bf16, tag="h5")
            nc.scalar.activation(h5[:], outp[:], AF.Relu)
            tp = psum_m.tile([P, P], bf16, tag="m")
            nc.tensor.transpose(tp[0:C, :], h5[:], ident[:])
            h5T = hpool.tile([K1, P], bf16, tag="h5T")
            nc.scalar.copy(h5T[0:C, :], tp[0:C, :])
            nc.vector.memset(h5T[C:C + 1, :], 1.0)
            o1 = psum_m.tile([P, 512], f32, tag="m")
            nc.tensor.matmul(o1[:, 0:O1], h5T[:], v1t[:], start=True, stop=True)
            o1r = hpool.tile([P, O1], bf16, tag="o1r")
            nc.scalar.activation(o1r[:], o1[:, 0:O1], AF.Relu)
            tp1 = psum_m.tile([P, P], bf16, tag="m")
            nc.tensor.transpose(tp1[0:O1, :], o1r[:], ident[:])
            o1T = hpool.tile([O1 + 1, P], bf16, tag="o1T")
            nc.scalar.copy(o1T[0:O1, :], tp1[0:O1, :])
            nc.vector.memset(o1T[O1:O1 + 1, :], 1.0)
            o2 = psum_m.tile([P, 512], f32, tag="m")
            nc.tensor.matmul(o2[:, 0:O2], o1T[:], v2t[:], start=True, stop=True)
            o2r = hpool.tile([P, O2], bf16, tag="o2r")
            nc.scalar.activation(o2r[:], o2[:, 0:O2], AF.Relu)
            tp2 = psum_m.tile([P, P], bf16, tag="m")
            nc.tensor.transpose(tp2[0:O2, :], o2r[:], ident[:])
            o2T = hpool.tile([O2 + 1, P], bf16, tag="o2T")
            nc.vector.memset(o2T[:], 1.0)  # row O2 keeps the 1.0 bias input
            nc.scalar.copy(o2T[0:O2, :], tp2[0:O2, :])
            o3 = psum_m.tile([P, 512], f32, tag="m")
            nc.tensor.matmul(o3[:, 0:O3], o2T[:], v3t[:], start=True, stop=True)
            ex = small.tile([P, O3], f32, tag="ex")
            ssum = small.tile([P, 1], f32, tag="ssum")
            nc.scalar.activation(ex[:], o3[:, 0:O3], AF.Exp, accum_out=ssum[:])
            lss = small.tile([P, 1], f32, tag="lss")
            nc.scalar.activation(lss[:], ssum[:], AF.Ln)
            fin = hpool.tile([P, O3], f32, tag="fin")
            nc.vector.tensor_scalar(fin[:], o3[:, 0:O3], lss[:], None,
                                    OP.subtract)
            sz = min(P, npc - P * w)
            nc.sync.dma_start(out_dram[P * w:P * w + sz, :], fin[0:sz, :])

        # ------------------------------------------------------------------
        def finalize_window(l, w, agg):
            rs = small.tile([P, H], f32, tag="rs")
            nc.vector.reciprocal(rs[:], agg[:, HK:HK + H])
            aggn = aggn_p.tile([P, HK], bf16, tag="aggn")
            nc.vector.tensor_tensor(
                aggn[:].rearrange("p (h c) -> p h c", c=C),
                agg[:, 0:HK].rearrange("p (h c) -> p h c", c=C),
                rs[:].unsqueeze(2).to_broadcast([P, H, C]),
                OP.mult)
            tps = psum_m.tile([P, NCH, P], bf16, tag="m")
            for j in range(NCH):
                jsz = min(P, HK - j * P)
                nc.tensor.transpose(tps[0:jsz, j, :], aggn[:, ds(j * P, jsz)],
                                    ident[:])
            aggnT = aggn_p.tile([P, NCH, P], bf16, tag="aggnT")
            nc.scalar.copy(aggnT[:, 0:NCH - 1, :], tps[:, 0:NCH - 1, :])
            lastsz = HK - (NCH - 1) * P
            nc.scalar.copy(aggnT[0:lastsz, NCH - 1, :],
                           tps[0:lastsz, NCH - 1, :])
            outp = psum_m.tile([P, C], f32, tag="m")
            for j in range(NCH):
                jsz = min(P, HK - j * P)
                nc.tensor.matmul(outp[:], aggnT[0:jsz, j, :],
                                 wbig[l][0:jsz, j, :],
                                 start=(j == 0), stop=False)
            nc.tensor.matmul(outp[:], ones_row[:], crow[l][:],
                             start=False, stop=True)
            if l < L - 1:
                write_h(w, outp)
            else:
                out_mlp(w, outp)

        # ------------------------------------------------------------------
        def gat_layer(l):
            cur = {}
            for st in range(n_st):
                siw = idxp.tile([P, STS], i32, tag="siw")
                nc.sync.dma_start(siw[:], srcw_in[st])
                hsE = gath.tile([P, STS, P], bf16, tag="hsE")
                for t in range(min(STS, n_tiles - st * STS)):
                    nc.gpsimd.indirect_dma_start(
                        out=hsE[:, t, :], out_offset=None, in_=hg[:],
                        in_offset=bass.IndirectOffsetOnAxis(
                            ap=siw[:, t:t + 1], axis=0))
                for t in range(STS):
                    gt = st * STS + t
                    if gt >= n_tiles:
                        break
                    w = win_of[gt]
                    first = (gt == 0) or (win_of[gt - 1] != w)
                    last = (gt == n_tiles - 1) or (win_of[gt + 1] != w)
                    if first:
                        agg_new = psum_agg.tile([P, AGG], f32, tag="agg")
                        cur["agg"] = agg_new
                        # xr side for this dst window: V_w = h_win @ Wy_hd,
                        # expanded per edge via the one-hot S01T lhsT below.
                        hwin = hpool.tile([P, P], bf16, tag="hwin")
                        nc.vector.memset(hwin[:], 0.0)
                        wsz = min(P, npc - P * w)
                        nc.sync.dma_start(hwin[0:wsz, :],
                                          hloc[P * w:P * w + wsz, :])
                        twT = psum_t.tile([P, 2 * P], bf16, tag="tT")
                        nc.tensor.transpose(twT[0:K1, 0:P], hwin[:, 0:K1],
                                            ident[:])
                        hwT = hpool.tile([K1, P], bf16, tag="hwT")
                        nc.scalar.copy(hwT[0:K1, :], twT[0:K1, 0:P])
                        vw0 = psum_y0.tile([P, Y0], f32, tag="y0")
                        nc.tensor.matmul(vw0[:], hwT[0:K1, :],
                                         wy_hd[l][:, 0:Y0],
                                         start=True, stop=True)
                        vw1 = psum_y1.tile([P, Y1C], f32, tag="y1")
                        nc.tensor.matmul(vw1[:], hwT[0:K1, :],
                                         wy_hd[l][:, Y0:YC],
                                         start=True, stop=True)
                        vw_new = gath.tile([P, YC], bf16, tag="vw")
                        nc.scalar.copy(vw_new[:, 0:Y0], vw0[:])
                        nc.scalar.copy(vw_new[:, Y0:YC], vw1[:])
                        cur["vw"] = vw_new
                    agg = cur["agg"]
                    vw = cur["vw"]

                    st_t = spool.tile([P, P], bf16, tag="s_t")
                    nc.sync.dma_start(st_t[:], s_in[gt])
                    s01t_t = spool.tile([P, P], bf16, tag="s01t_t")
                    nc.sync.dma_start(s01t_t[:], s01t_in[gt])

                    # transpose gathered src rows into lhsT layout [65, 128]
                    tT = psum_t.tile([P, 2 * P], bf16, tag="tT")
                    nc.tensor.transpose(tT[0:K1, 0:P], hsE[:, t, 0:K1],
                                        ident[:])
                    lhs = lhsp.tile([K1, P], bf16, tag="lhs")
                    nc.scalar.copy(lhs[0:K1, :], tT[0:K1, 0:P])
                    hsT_sl = lhs[0:K1, 0:P]
                    y0 = psum_y0.tile([P, Y0], f32, tag="y0")
                    nc.tensor.matmul(y0[:], hsT_sl, wy_hs[l][:, 0:Y0],
                                     start=True, stop=False)
                    nc.tensor.matmul(y0[:], s01t_t[:], vw[:, 0:Y0],
                                     start=False, stop=True)
                    y1 = psum_y1.tile([P, Y1C], f32, tag="y1")
                    nc.tensor.matmul(y1[:], hsT_sl, wy_hs[l][:, Y0:YC],
                                     start=True, stop=False)
                    nc.tensor.matmul(y1[:], s01t_t[:], vw[:, Y0:YC],
                                     start=False, stop=True)

                    ay = aypool.tile([P, Y0], bf16, tag="ay")
                    nc.scalar.activation(ay[:], y0[:], AF.Abs)
                    red0 = small.tile([P, Y0G], f32, tag="red0")
                    nc.vector.tensor_reduce(
                        red0[:], ay[:].rearrange("p (g s) -> p g s", s=Gm),
                        axis=AX.X, op=OP.add)
                    red1 = small.tile([P, 2], f32, tag="red1")
                    nc.vector.tensor_reduce(
                        red1[:],
                        y1[:, 0:2 * Gm].rearrange("p (g s) -> p g s", s=Gm),
                        axis=AX.X, op=OP.add, apply_absolute_value=True)
                    logits = small.tile([P, H], f32, tag="logits")
                    r0 = red0[:].rearrange("p (h g) -> p h g", g=2)
                    nc.vector.tensor_tensor(logits[:, 0:H - 1],
                                            r0[:, :, 0], r0[:, :, 1],
                                            OP.subtract)
                    nc.vector.tensor_tensor(logits[:, H - 1:H],
                                            red1[:, 0:1], red1[:, 1:2],
                                            OP.subtract)
                    lg2 = small.tile([P, H], f32, tag="lg2")
                    nc.vector.tensor_tensor(lg2[:], logits[:],
                                            y1[:, 2 * Gm:2 * Gm + H], OP.add)
                    wt = wtp.tile([P, AGG], bf16, tag="wt")
                    nc.scalar.activation(wt[:, HK:HK + H], lg2[:], AF.Exp)
                    nc.vector.tensor_tensor(
                        wt[:, 0:HK].rearrange("p (h c) -> p h c", c=C),
                        hsE[:, t:t + 1, 0:C].to_broadcast([P, H, C]),
                        wt[:, HK:HK + H].unsqueeze(2).to_broadcast([P, H, C]),
                        OP.mult)
                    nc.tensor.matmul(agg[:], st_t[:], wt[:],
                                     start=first, stop=last)
                    if last:
                        finalize_window(l, w, agg)

        # ------------------------------ main ------------------------------
        input_mlp()
        grp = [list(range(NCC))]
        for l in range(L):
            if meta.get("no_collectives"):
                nc.sync.dma_start(hg[0:npc, :], hloc[:])
            else:
                nc.gpsimd.collective_compute(
                    "AllGather", OP.bypass, replica_groups=grp,
                    ins=[hloc[:].opt()], outs=[hg[:].opt()])
            gat_layer(l)

    nc.compile()
    return nc


# ---------------------------------------------------------------------------
# entry point
# ---------------------------------------------------------------------------

LAST_RESULT = None


def kernel(**inputs) -> np.ndarray:
    import os
    from concourse import bass_utils
    global LAST_RESULT

    nc_cores = NC_DEFAULT
    in_maps, meta = _host_prep(inputs, nc_cores)
    if "nc" not in _CACHE:
        _CACHE["nc"] = _build(meta)
    nc = _CACHE["nc"]

    kw = {}
    if os.environ.get("KERNEL_TRACE"):
        kw = dict(trace=True, trace_cores=list(range(nc_cores)))
    res = bass_utils.run_bass_kernel_spmd(
        nc, in_maps, core_ids=list(range(nc_cores)), **kw)
    LAST_RESULT = res
    out = np.concatenate([r["out"] for r in res.results], axis=0)
    return np.asarray(out, np.float32)


if __name__ == "__main__":
    import reference
    inputs = {k: np.asarray(v) for k, v in reference.setup_inputs().items()}
    got = kernel(**inputs)
    print("kernel output", got.shape, got.dtype)
